# revision 32
# baseline (speedup 1.0000x reference)
"""GPT2 attention (B=2,S=2048,E=1024,H=16) on 8 NeuronCores.

Sharding: core c -> batch b=c//4, head-group g=c%4 (4 heads, d'=256 cols).

Schedule (v3): ACT-paced software pipeline. Per q-chunk (512 q), the two
head pairs run sequential k-pair chains; attn@V lags one step behind exp;
QKV-proj and c_proj matmuls are drip-fed as PE filler between steps.
Scores matmuls use zero-padded per-head K stationaries (kte/kto) so every
matmul runs in 128x128 mode (no PE tiling-mode-switch drains). Causal
structure is trimmed at 128-col granularity. Input DMAs are spread across
engine queues; c_proj partials are written bf16 and summed on host.
"""

import numpy as np

import concourse.bass as bass
import concourse.mybir as mybir
import concourse.tile as tile
from concourse import bacc
from concourse.bass_utils import run_bass_kernel_spmd

B, S, E, H = 2, 2048, 1024, 16
HD = 64           # head dim
HPC = 4           # heads per core
DP = HPC * HD     # 256 d' columns per core
NQC = 4           # q-chunks of 512
NST = S // 128    # 16 s-tiles
NET = E // 128    # 8 E-tiles

f32 = mybir.dt.float32
bf16 = mybir.dt.bfloat16
FT = mybir.ActivationFunctionType

_CACHED = {}


def build_nc():
    nc = bacc.Bacc("TRN2", target_bir_lowering=False, debug=False,
                   enable_asserts=False, num_devices=8)

    xT = nc.dram_tensor("xT", [E, S], bf16, kind="ExternalInput")
    wqk = nc.dram_tensor("wqk", [E, 2 * DP], bf16, kind="ExternalInput")
    bqk = nc.dram_tensor("bqk", [128, 4], f32, kind="ExternalInput")
    wv = nc.dram_tensor("wv", [E, 260], bf16, kind="ExternalInput")
    vb = nc.dram_tensor("vb", [128, 260], f32, kind="ExternalInput")
    wp = nc.dram_tensor("wp", [DP, E], bf16, kind="ExternalInput")
    mtri = nc.dram_tensor("mtri", [128, 128], bf16, kind="ExternalInput")
    outp = nc.dram_tensor("outp", [S, E], bf16, kind="ExternalOutput")

    with tile.TileContext(nc) as tc:
        with (
            nc.allow_low_precision("bf16 data with fp32 psum accumulation"),
            tc.tile_pool(name="consts", bufs=1) as consts,
            tc.tile_pool(name="acts", bufs=1) as acts,
            tc.tile_pool(name="slabs", bufs=6) as slabs,
            tc.tile_pool(name="small", bufs=4) as small,
            tc.tile_pool(name="outs", bufs=4) as outs,
            tc.tile_pool(name="scps", bufs=2, space="PSUM") as scps,
            tc.tile_pool(name="otps", bufs=2, space="PSUM") as otps,
            tc.tile_pool(name="fps", bufs=2, space="PSUM") as fps,
        ):
            # ---- inputs in on sync+gpsimd rings only: DMA issues occupy
            # the issuing engine's queue, and the scalar queue must stay
            # clear for the exp stream (the critical ACT path).
            bqk_sb = consts.tile([128, 4], f32, tag="bqk")
            nc.sync.dma_start(bqk_sb[:], bqk[:, :])
            # xT split: wave0 = columns 0-511 (all that the sc=0
            # projections and V s-tiles 0-3 need -> attention starts after
            # ~2MB instead of 5MB), rest = columns 512-2047. wqk[t] and
            # xt0[t] are interleaved pairwise per ring so the K-proj matmul
            # for k-tile t can start as soon as its own pair lands.
            wqkk_sb = [None] * NET
            wqkq_sb = [None] * NET
            xt0_sb, xtr_sb = [None] * NET, []
            for t in range(NET):
                eng = nc.gpsimd if t % 2 == 0 else nc.sync
                wk = consts.tile([128, DP], bf16, tag=f"wqkk{t}",
                                 name=f"wk{t}")
                eng.dma_start(wk[:], wqk[t * 128:(t + 1) * 128, DP:2 * DP])
                wqkk_sb[t] = wk
                xw = consts.tile([128, 512], bf16, tag=f"xt{t}w0",
                                 name=f"xw{t}")
                eng.dma_start(xw[:], xT[t * 128:(t + 1) * 128, 0:512])
                xt0_sb[t] = xw
            for t in range(NET):
                eng = nc.gpsimd if t % 2 == 0 else nc.sync
                wq = consts.tile([128, DP], bf16, tag=f"wqkq{t}",
                                 name=f"wqq{t}")
                eng.dma_start(wq[:], wqk[t * 128:(t + 1) * 128, 0:DP])
                wqkq_sb[t] = wq
            wv_sb = []
            for t in range(NET):
                wvt = consts.tile([128, 260], bf16, tag=f"wv{t}")
                nc.gpsimd.dma_start(wvt[:], wv[t * 128:(t + 1) * 128, :])
                wv_sb.append(wvt)
            vb_sb = consts.tile([128, 260], f32, tag="vb")
            nc.sync.dma_start(vb_sb[:], vb[:, :])
            mtri_sb = consts.tile([128, 128], bf16, tag="mtri")
            nc.sync.dma_start(mtri_sb[:], mtri[:, :])
            for t in range(NET):
                xr = consts.tile([128, 1536], bf16, tag=f"xt{t}r")
                eng = nc.gpsimd if t % 2 == 0 else nc.sync
                eng.dma_start(xr[:], xT[t * 128:(t + 1) * 128, 512:2048])
                xtr_sb.append(xr)
            wp_sb = []
            for t in range(2):
                wpt = consts.tile([128, E], bf16, tag=f"wp{t}")
                nc.scalar.dma_start(wpt[:], wp[t * 128:(t + 1) * 128, :])
                wp_sb.append(wpt)

            def xt_cols(kt, c0, c1):
                """xT[kt-tile][:, c0:c1] from the wave-split tiles."""
                if c1 <= 512:
                    return xt0_sb[kt][:, c0:c1]
                return xtr_sb[kt][:, c0 - 512:c1 - 512]

            # ---- persistent activations ----
            v_sb = [acts.tile([128, 260], bf16, tag=f"v{st}", name=f"v{st}")
                    for st in range(NST)]
            qt_sb = [acts.tile([128, S], bf16, tag=f"qt{t}", name=f"qt{t}")
                     for t in range(2)]
            kte_sb = [acts.tile([128, S], bf16, tag=f"kte{t}", name=f"kte{t}")
                      for t in range(2)]
            kto_sb = [acts.tile([128, S], bf16, tag=f"kto{t}", name=f"kto{t}")
                      for t in range(2)]
            attnT_sb = [acts.tile([128, S], bf16, tag=f"attnT{t}",
                                  name=f"attnT{t}") for t in range(2)]
            scr_sb = acts.tile([1, 8], f32, tag="scr", name="scr")

            # zero the padding halves of the K tiles (read by every scores
            # matmul; the zero rows nullify the other head's moving rows)
            for t in range(2):
                nc.vector.memset(kte_sb[t][64:128, :], 0.0)
                nc.vector.memset(kto_sb[t][0:64, :], 0.0)
            # preload the exp table set while DMAs stream
            nc.scalar.activation(scr_sb[0:1, 0:4], bqk_sb[0:1, 0:4], FT.Exp)
            # warm the PE HAM clock gate during the input-DMA wait: dummy
            # matmuls on an uninitialized tile keep the PE busy >3.4us so
            # the real projections start at 2.4GHz instead of 1.2GHz
            wps_warm = fps.tile([128, 512], f32, tag="fp", name="wps_warm")
            for i in range(18):
                nc.tensor.matmul(wps_warm[:], attnT_sb[0][:, 0:128],
                                 attnT_sb[0][:, 0:512],
                                 start=(i == 0), stop=(i == 17))

            # ================= filler units (PE work drip-fed) ==========
            def emit_kq(t, sc):
                """Q or K proj column chunk sc (+bias).

                t in 0..1 -> Q tile t; t in 2..3 -> K pair t-2 (split into
                zero-padded per-head tiles kte/kto)."""
                qps = fps.tile([128, 512], f32, tag="fp", name="qps")
                for kt in range(NET):
                    if t < 2:
                        wsl = wqkq_sb[kt][:, t * 128:(t + 1) * 128]
                    else:
                        wsl = wqkk_sb[kt][:, (t - 2) * 128:(t - 1) * 128]
                    nc.tensor.matmul(
                        qps[:],
                        wsl,
                        xt_cols(kt, sc * 512, (sc + 1) * 512),
                        start=(kt == 0), stop=(kt == NET - 1),
                    )
                cs = slice(sc * 512, (sc + 1) * 512)
                if t < 2:
                    nc.vector.tensor_scalar_add(
                        qt_sb[t][:, cs], qps[:], bqk_sb[:, t:t + 1])
                else:
                    tq = t - 2
                    nc.vector.tensor_scalar_add(
                        kte_sb[tq][0:64, cs], qps[0:64, :],
                        bqk_sb[0:64, t:t + 1])
                    nc.vector.tensor_scalar_add(
                        kto_sb[tq][64:128, cs], qps[64:128, :],
                        bqk_sb[64:128, t:t + 1])

            def emit_v(st):
                """V_aug[s-tile st] = x @ Wv_aug + vb (ones col via vb)."""
                vps = fps.tile([128, 512], f32, tag="fp", name="vps")
                for kt in range(NET):
                    nc.tensor.matmul(
                        vps[:, 0:260],
                        xt_cols(kt, st * 128, (st + 1) * 128),
                        wv_sb[kt][:],
                        start=(kt == 0), stop=(kt == NET - 1),
                    )
                nc.vector.tensor_add(v_sb[st][:], vps[:, 0:260], vb_sb[:])

            def emit_cp(st):
                """c_proj partial for s-tile st -> DRAM (bf16)."""
                ob = outs.tile([128, E], bf16, tag="ob", name="ob")
                for nchk in range(2):
                    cps = fps.tile([128, 512], f32, tag="fp", name="cps")
                    for kt2 in range(2):
                        nc.tensor.matmul(
                            cps[:],
                            attnT_sb[kt2][:, st * 128:(st + 1) * 128],
                            wp_sb[kt2][:, nchk * 512:(nchk + 1) * 512],
                            start=(kt2 == 0), stop=(kt2 == 1),
                        )
                    nc.vector.tensor_copy(
                        ob[:, nchk * 512:(nchk + 1) * 512], cps[:])
                nc.sync.dma_start(outp[st * 128:(st + 1) * 128, :], ob[:])

            queue = []
            emitted = set()

            def push(kind, a, b=None):
                queue.append((kind, a, b))

            def do_emit(u):
                if u in emitted:
                    return
                emitted.add(u)
                kind, a, b = u
                if kind == "KQ":
                    emit_kq(a, b)
                elif kind == "V":
                    emit_v(a)
                else:
                    emit_cp(a)

            def ensure(u):
                if u not in emitted:
                    do_emit(u)

            def pop_one():
                while queue:
                    u = queue.pop(0)
                    if u not in emitted:
                        do_emit(u)
                        return

            # ================= attention units ==========================
            def emit_scores(qc, pair, kp, sps):
                qt_ap = qt_sb[pair]
                for half in range(2):
                    kt = 2 * kp + half
                    di = kt - 4 * qc
                    q0 = max(di, 0) * 128  # first valid q col in chunk
                    for i, ksb in enumerate((kte_sb[pair], kto_sb[pair])):
                        nc.tensor.matmul(
                            sps[i][:, half * 512 + q0:(half + 1) * 512],
                            ksb[:, kt * 128:(kt + 1) * 128],
                            qt_ap[:, qc * 512 + q0:(qc + 1) * 512],
                            start=True, stop=True,
                        )

            def emit_exp(qc, pair, kp, sps, slbs):
                for i in range(2):
                    diag = 2 * kp >= 4 * qc
                    if not diag:
                        nc.scalar.activation(slbs[i][:], sps[i][:], FT.Exp)
                    else:
                        for half in range(2):
                            kt = 2 * kp + half
                            q0 = max(kt - 4 * qc, 0) * 128
                            c0 = half * 512 + q0
                            c1 = (half + 1) * 512
                            nc.scalar.activation(
                                slbs[i][:, c0:c1], sps[i][:, c0:c1], FT.Exp)

            def emit_mask(qc, pair, kp, slbs):
                for half in range(2):
                    kt = 2 * kp + half
                    di = kt - 4 * qc
                    if di >= 0:
                        c0 = half * 512 + di * 128
                        for i in range(2):
                            nc.vector.tensor_mul(
                                slbs[i][:, c0:c0 + 128],
                                slbs[i][:, c0:c0 + 128], mtri_sb[:])

            def emit_av(qc, pair, kp, slbs, ots, nkt):
                for half in range(2):
                    kt = 2 * kp + half
                    di = kt - 4 * qc
                    q0 = max(di, 0) * 128
                    ensure(("V", kt, None))
                    for i in range(2):
                        hl = 2 * pair + i
                        nc.tensor.matmul(
                            ots[i][:, q0:512],
                            v_sb[kt][:, 65 * hl:65 * hl + 65],
                            slbs[i][:, half * 512 + q0:(half + 1) * 512],
                            start=(kt == 0), stop=(kt == nkt - 1),
                        )

            def emit_norm_rz(pair, ots):
                """Z -> 1/Z -> broadcast [64,512] for both heads."""
                zrows, rzs, sbbs = [], [], []
                for i in range(2):
                    zr = small.tile([1, 512], f32, tag="zrow", name="zrow")
                    nc.vector.tensor_copy(zr[:], ots[i][64:65, :])
                    zrows.append(zr)
                for i in range(2):
                    rz = small.tile([1, 512], f32, tag="rz", name="rz")
                    nc.vector.reciprocal_approx_fast(rz[:], zrows[i][:])
                    rzs.append(rz)
                for i in range(2):
                    sbb = small.tile([64, 512], f32, tag="sbb", name="sbb")
                    nc.gpsimd.partition_broadcast(sbb[:], rzs[i][0:1, :])
                    sbbs.append(sbb)
                return sbbs

            def emit_norm(qc, pair, ots):
                sbbs = emit_norm_rz(pair, ots)
                for i in range(2):
                    hl = 2 * pair + i
                    po = (hl % 2) * 64
                    nc.vector.tensor_mul(
                        attnT_sb[hl // 2][po:po + 64,
                                          qc * 512:(qc + 1) * 512],
                        ots[i][0:64, :], sbbs[i][:])

            # ================= the schedule =============================
            for u in [("KQ", 3, 0), ("KQ", 1, 0), ("V", 0, None),
                      ("V", 1, None), ("V", 2, None), ("V", 3, None),
                      ("KQ", 2, 1), ("KQ", 0, 1), ("V", 4, None),
                      ("V", 5, None), ("V", 6, None), ("V", 7, None),
                      ("KQ", 3, 1), ("KQ", 1, 1),
                      ("KQ", 2, 2), ("KQ", 0, 2),
                      ("V", 8, None), ("V", 9, None), ("V", 10, None),
                      ("V", 11, None),
                      ("KQ", 3, 2), ("KQ", 1, 2),
                      ("KQ", 2, 3), ("KQ", 0, 3),
                      ("V", 12, None), ("V", 13, None), ("V", 14, None),
                      ("V", 15, None),
                      ("KQ", 3, 3), ("KQ", 1, 3)]:
                push(*u)

            # head: minimal K/Q proj for qc0 pair0, with the K and Q
            # matmuls interleaved at k-tile granularity so both finish as
            # soon as the last xt0 tile lands (Q does not serialize after K)
            emitted.add(("KQ", 2, 0))
            emitted.add(("KQ", 0, 0))
            ps_k = fps.tile([128, 512], f32, tag="fp", name="ps_k")
            ps_q = fps.tile([128, 512], f32, tag="fp", name="ps_q")
            for kt in range(NET):
                nc.tensor.matmul(
                    ps_k[:], wqkk_sb[kt][:, 0:128], xt0_sb[kt][:],
                    start=(kt == 0), stop=(kt == NET - 1))
                nc.tensor.matmul(
                    ps_q[:], wqkq_sb[kt][:, 0:128], xt0_sb[kt][:],
                    start=(kt == 0), stop=(kt == NET - 1))
            nc.vector.tensor_scalar_add(
                kte_sb[0][0:64, 0:512], ps_k[0:64, :], bqk_sb[0:64, 2:3])
            nc.vector.tensor_scalar_add(
                kto_sb[0][64:128, 0:512], ps_k[64:128, :],
                bqk_sb[64:128, 2:3])
            nc.vector.tensor_scalar_add(
                qt_sb[0][:, 0:512], ps_q[:], bqk_sb[:, 0:1])

            steps = []
            for qc in range(NQC):
                nkt = 4 * qc + 4
                for pair in range(2):
                    for kp in range(nkt // 2):
                        steps.append((qc, pair, kp, nkt))

            prev = None          # (qc, pair, kp, nkt, slbs, ots)
            ots_cur = None
            for (qc, pair, kp, nkt) in steps:
                if kp == 0:
                    ensure(("KQ", 2 + pair, qc))
                    ensure(("KQ", pair, qc))
                    ots_cur = [otps.tile([65, 512], f32, tag="ot",
                                         name=f"ot{qc}_{pair}_{i}")
                               for i in range(2)]
                sps = [scps.tile([128, 1024], f32, tag="sp",
                                 name=f"sp{qc}_{pair}_{kp}_{i}")
                       for i in range(2)]
                slbs = [slabs.tile([128, 1024], bf16, tag="slab",
                                   name=f"sl{qc}_{pair}_{kp}_{i}")
                        for i in range(2)]
                if prev is not None:
                    pqc, ppair, pkp, pnkt, pslbs, pots = prev
                    emit_av(pqc, ppair, pkp, pslbs, pots, pnkt)
                    if pkp == pnkt // 2 - 1:  # pair chain finished
                        emit_norm(pqc, ppair, pots)
                        if ppair == 1:  # whole qc finished -> c_proj
                            for st in range(4 * pqc, 4 * pqc + 4):
                                push("CP", st, None)
                emit_scores(qc, pair, kp, sps)
                emit_exp(qc, pair, kp, sps, slbs)
                emit_mask(qc, pair, kp, slbs)
                pop_one()
                if (qc, pair, kp) == (NQC - 1, 1, nkt // 2 - 1):
                    # last step: queue warm-keeper matmuls now (the psum
                    # slot from two steps ago is already free) so the PE
                    # stays at full clock through the tail's 1/Z chain
                    wrm = scps.tile([128, 1024], f32, tag="sp", name="wrm")
                    for i in range(8):
                        nc.tensor.matmul(
                            wrm[:, 0:512], attnT_sb[0][:, 0:128],
                            attnT_sb[0][:, 0:512],
                            start=(i == 0), stop=False)
                prev = (qc, pair, kp, nkt, slbs, ots_cur)

            # tail: last pair's norm split per s-tile so c_proj pipelines;
            # the attnT_sb[0]-half (kt2=0) of each c_proj runs during the
            # 1/Z chain (it only depends on the pair-0 norm, long done)
            pqc, ppair, pkp, pnkt, pslbs, pots = prev
            emit_av(pqc, ppair, pkp, pslbs, pots, pnkt)
            while queue:
                pop_one()
            sbbs = emit_norm_rz(ppair, pots)
            sts = list(range(4 * pqc, 4 * pqc + 4))
            tail_ps = {}
            for sti, st in enumerate(sts[:3]):
                if sti < 2:
                    tp = scps.tile([128, 1024], f32, tag="sp",
                                   name=f"tp{st}")
                    tail_ps[st] = [tp[:, 0:512], tp[:, 512:1024]]
                else:
                    tail_ps[st] = [
                        fps.tile([128, 512], f32, tag="fp", name=f"tp{st}a"),
                        fps.tile([128, 512], f32, tag="fp", name=f"tp{st}b")]
                for nchk in range(2):
                    nc.tensor.matmul(
                        tail_ps[st][nchk][:],
                        attnT_sb[0][:, st * 128:(st + 1) * 128],
                        wp_sb[0][:, nchk * 512:(nchk + 1) * 512],
                        start=True, stop=False,
                    )
            for i in range(6):  # keep the PE warm until 1/Z lands
                nc.tensor.matmul(wrm[:, 0:512], attnT_sb[0][:, 0:128],
                                 attnT_sb[0][:, 0:512],
                                 start=False, stop=(i == 5))
            for sti, st in enumerate(sts):
                blk = slice(sti * 128, (sti + 1) * 128)
                for i in range(2):
                    hl = 2 * ppair + i
                    po = (hl % 2) * 64
                    nc.vector.tensor_mul(
                        attnT_sb[hl // 2][po:po + 64,
                                          st * 128:(st + 1) * 128],
                        pots[i][0:64, blk], sbbs[i][:, blk])
                if st in tail_ps:
                    ob = outs.tile([128, E], bf16, tag="ob", name="ob")
                    for nchk in range(2):
                        nc.tensor.matmul(
                            tail_ps[st][nchk][:],
                            attnT_sb[1][:, st * 128:(st + 1) * 128],
                            wp_sb[1][:, nchk * 512:(nchk + 1) * 512],
                            start=False, stop=True,
                        )
                        if nchk == 0:
                            nc.vector.tensor_copy(
                                ob[:, 0:512], tail_ps[st][nchk][:])
                        else:
                            nc.scalar.copy(
                                ob[:, 512:1024], tail_ps[st][nchk][:])
                    nc.sync.dma_start(
                        outp[st * 128:(st + 1) * 128, :], ob[:])
                else:
                    ob = outs.tile([128, E], bf16, tag="ob", name="ob")
                    for nchk in range(2):
                        cps = fps.tile([128, 512], f32, tag="fp",
                                       name="cps")
                        for kt2 in range(2):
                            nc.tensor.matmul(
                                cps[:],
                                attnT_sb[kt2][:, st * 128:(st + 1) * 128],
                                wp_sb[kt2][:, nchk * 512:(nchk + 1) * 512],
                                start=(kt2 == 0), stop=(kt2 == 1),
                            )
                        if nchk == 0:
                            nc.vector.tensor_copy(ob[:, 0:512], cps[:])
                        else:
                            nc.scalar.copy(ob[:, 512:1024], cps[:])
                    nc.sync.dma_start(
                        outp[st * 128:(st + 1) * 128, :], ob[:])

    nc.finalize()
    return nc


def _prep_inputs(hidden_states, w_attn, b_attn, w_proj, b_proj):
    hs = np.asarray(hidden_states, np.float32)
    wa = np.asarray(w_attn, np.float32)
    ba = np.asarray(b_attn, np.float32)
    wpj = np.asarray(w_proj, np.float32)

    import ml_dtypes
    bfl = ml_dtypes.bfloat16
    xTs = [np.ascontiguousarray(hs[b].T.astype(bfl)) for b in range(B)]
    mtri = (np.arange(128)[:, None] <= np.arange(128)[None, :]).astype(bfl)

    in_maps = []
    for c in range(8):
        b, g = c // 4, c % 4
        q0 = DP * g
        k0 = E + DP * g
        v0 = 2 * E + DP * g
        wqk = np.concatenate(
            [wa[:, q0:q0 + DP] * 0.125, wa[:, k0:k0 + DP]], axis=1).astype(bfl)
        bqk = np.zeros((128, 4), np.float32)
        bqk[:, 0] = ba[q0:q0 + 128] * 0.125
        bqk[:, 1] = ba[q0 + 128:q0 + 256] * 0.125
        bqk[:, 2] = ba[k0:k0 + 128]
        bqk[:, 3] = ba[k0 + 128:k0 + 256]
        wv = np.zeros((E, 260), bfl)
        vb = np.zeros((128, 260), np.float32)
        for h in range(HPC):
            wv[:, 65 * h:65 * h + 64] = \
                wa[:, v0 + 64 * h:v0 + 64 * h + 64].astype(bfl)
            vb[:, 65 * h:65 * h + 64] = ba[v0 + 64 * h:v0 + 64 * h + 64]
            vb[:, 65 * h + 64] = 1.0
        wp = np.ascontiguousarray(wpj[DP * g:DP * (g + 1), :].astype(bfl))
        in_maps.append({
            "xT": xTs[b],
            "wqk": np.ascontiguousarray(wqk),
            "bqk": bqk,
            "wv": wv,
            "vb": vb,
            "wp": wp,
            "mtri": mtri,
        })
    return in_maps


def run(trace=False, **inputs):
    if "nc" not in _CACHED:
        _CACHED["nc"] = build_nc()
    nc = _CACHED["nc"]
    in_maps = _prep_inputs(**inputs)
    res = run_bass_kernel_spmd(nc, in_maps, list(range(8)), trace=trace)
    b_proj = np.asarray(inputs["b_proj"], np.float32)
    out = np.empty((B, S, E), np.float32)
    for b in range(B):
        acc = res.results[4 * b]["outp"].astype(np.float32)
        for g in range(1, 4):
            acc = acc + res.results[4 * b + g]["outp"].astype(np.float32)
        out[b] = acc + b_proj
    return out, res


def kernel(**inputs):
    out, _ = run(trace=False, **inputs)
    return out
